# revision 19
# baseline (speedup 1.0000x reference)
"""Block-sparse linear kernel for Trainium2 (8 NeuronCores, raw Bass/bacc).

Computes out[n, ob*BS:(ob+1)*BS] += x[n, ib*BS:(ib+1)*BS] @ W[k] for each
nonzero block k with indices (ob, ib), plus bias — data-parallel over the
flattened row dim N across 8 cores (weights replicated, indices baked into
the schedule host-side).

Host-side schedule (same as the Tile baseline):
  - Group input-blocks (ibs) into *families* with identical sets of
    output-blocks (obs); for the canonical every-10th-block pattern the
    families are the 5 residue classes mod 5 (disjoint, no zero fill).
  - Pair ibs within a family: each pair is one K=128 stationary operand
    (two 64-feature x slices, transposed host-side), streaming a
    [128, n_obs*64] stacked-weight moving operand -> full PE utilization.
  - One combined input tensor holds stacked weights and transposed x
    slices in exact consumption order; a single sequential DMA stream
    delivers data just-in-time.

Device module (this version): raw bacc, no TileContext.  The Tile
epilogue (per-semaphore EVSEM chains, ~7.4us with 254 sems) and its
scheduling slack were the dominant overheads in the 44.6us baseline.
Here the whole pipeline is synchronized with 6 manual semaphores:
  s_in   +16 per input DMA chunk (HWDGE completion)
  s_ws   +1 when the warmup operand tile is memset
  s_mm   +1 per PSUM accumulation group finished (PE order)
  s_evA  +1 per ACT eviction        s_evB  +1 per DVE eviction
  s_st   +16 per output store DMA
PE waits on s_in at chunk-boundary tasks and on s_ev* for PSUM buffer
reuse; ACT/DVE evict alternating groups (last segment split half/half
for a short tail); SP issues all input loads up front, then stores as
evictions land.  bf16 in/out as before (rel err ~2.9e-3).
"""

import os
import numpy as np
import ml_dtypes
from bisect import bisect_left
from collections import defaultdict

from concourse import bass_utils, bacc, mybir

N_CORES = 8
P = 128            # partitions / row-tile size
SEG_MAX_OBS = int(os.environ.get("KSEG", "16"))  # blocks per psum segment
F32R = mybir.dt.float32r
F32 = mybir.dt.float32
BF16 = mybir.dt.bfloat16

KDTYPE = os.environ.get("KDTYPE", "bf16")
DT_IN = BF16 if KDTYPE == "bf16" else F32R
NP_IN = ml_dtypes.bfloat16 if KDTYPE == "bf16" else np.float32
KOUT = os.environ.get("KOUT", "bf16")
DT_OUT = BF16 if KOUT == "bf16" else F32
NP_OUT = ml_dtypes.bfloat16 if KOUT == "bf16" else np.float32

_CACHE = {}
LAST_RESULT = None


def _build_schedule(N, F, OUT_F, BS, out_idx, in_idx):
    """Pure-index schedule: families, pairs, segments, layouts."""
    n_ib = F // BS
    n_ob = OUT_F // BS
    assert F % BS == 0 and OUT_F % BS == 0

    wslots = defaultdict(list)
    for k, (ob, ib) in enumerate(zip(out_idx, in_idx)):
        ob, ib = int(ob), int(ib)
        assert 0 <= ob < n_ob and 0 <= ib < n_ib
        wslots[(ob, ib)].append(k)

    obs_by_ib = defaultdict(set)
    for (ob, ib) in wslots:
        obs_by_ib[ib].add(ob)

    fam_map = defaultdict(list)
    for ib in sorted(obs_by_ib):
        fam_map[frozenset(obs_by_ib[ib])].append(ib)
    families = [(sorted(obs), ibs) for obs, ibs in fam_map.items()]

    parent = {}

    def find(a):
        while parent[a] != a:
            parent[a] = parent[parent[a]]
            a = parent[a]
        return a

    for obs, _ in families:
        for ob in obs:
            parent.setdefault(ob, ob)
        r = find(obs[0])
        for ob in obs[1:]:
            parent[find(ob)] = r
    sf_map = defaultdict(lambda: {"obs": set(), "fams": []})
    for obs, ibs in families:
        root = find(obs[0])
        sf_map[root]["obs"].update(obs)
        sf_map[root]["fams"].append((obs, ibs))
    superfams = sorted(sf_map.values(), key=lambda s: min(s["obs"]))

    # order superfamilies: smallest stream first (early PE start), smallest
    # last (short tail); big ones in the middle.
    def sf_cols(sf):
        u = sum((len(ibs) + 1) // 2 for _, ibs in sf["fams"])
        return u * len(sf["obs"]) * BS

    if len(superfams) > 2:
        rest = sorted(superfams, key=sf_cols)
        first, last = rest[1], rest[0]
        mid = rest[2:]
        superfams = [first] + mid + [last]

    xt_tiles = []
    singles = []
    fam_units = defaultdict(list)
    fam_id = 0
    for sf in superfams:
        for obs, ibs in sf["fams"]:
            key = fam_id
            for i in range(0, len(ibs) - 1, 2):
                t = len(xt_tiles)
                xt_tiles.append([(0, ibs[i]), (64, ibs[i + 1])])
                fam_units[key].append((t, 0, 128, (ibs[i], ibs[i + 1])))
            if len(ibs) % 2:
                singles.append((key, ibs[-1]))
            fam_id += 1
    for j in range(0, len(singles), 2):
        t = len(xt_tiles)
        entries = [(0, singles[j][1])]
        fam_units[singles[j][0]].append((t, 0, 64, (singles[j][1],)))
        if j + 1 < len(singles):
            entries.append((64, singles[j + 1][1]))
            fam_units[singles[j + 1][0]].append((t, 64, 64, (singles[j + 1][1],)))
        xt_tiles.append(entries)

    n_pad = (-N) % (N_CORES * P)
    rows_per_core = (N + n_pad) // N_CORES
    rt_count = rows_per_core // P
    Nc = rows_per_core

    # segments + combined-input layout + out layout
    segments = []
    in_blocks = []
    xt_off = {}
    in_cols = 0
    out_cols = 0
    cuts = []
    fid = 0
    for sfi, sf in enumerate(superfams):
        sf_obs = sorted(sf["obs"])
        units = []
        for obs, ibs in sf["fams"]:
            units.append((fid, tuple(obs)))
            fid += 1
        for s0 in range(0, len(sf_obs), SEG_MAX_OBS):
            seg_obs = sf_obs[s0:s0 + SEG_MAX_OBS]
            L = len(seg_obs) * BS
            tasks = []
            all_units = []
            for key, fobs in units:
                for (t, rb, kr, uibs) in fam_units[key]:
                    all_units.append((t, rb, kr, uibs))
            for ui, (t, rb, kr, uibs) in enumerate(all_units):
                wc = in_cols
                in_blocks.append((wc, "w", rb, uibs, seg_obs))
                in_cols += L
                if t not in xt_off:
                    xt_off[t] = in_cols
                    in_blocks.append((in_cols, "x", t, None, None))
                    in_cols += Nc
                for c0 in range(0, L, 512):
                    c1 = min(c0 + 512, L)
                    tasks.append((c0, c1, xt_off[t], rb, kr, wc + c0,
                                  ui == 0, ui == len(all_units) - 1))
                if len(cuts) == 0 and len(segments) == 0 and ui == 0:
                    cuts.append(in_cols)   # first chunk: unit0 (+ its xt)
            segments.append({"out_base": out_cols, "n_obs": len(seg_obs),
                             "obs": seg_obs, "tasks": tasks})
            out_cols += L
    cuts.append(in_cols)

    # chunking: small chunks at the head (the ~1-2us DMA completion receipt
    # latency otherwise stalls the PE while it still tracks the stream),
    # bigger chunks once the PE has fallen behind the load stream.
    CHUNK_COLS = int(os.environ.get("KCHUNK", "3400"))
    CHUNK1_COLS = int(os.environ.get("KCHUNK1", "1100"))
    HEAD_COLS = int(os.environ.get("KHEAD", "3000"))
    block_edges = sorted({b[0] for b in in_blocks} | {in_cols})
    # extra-fine edges inside the first unit so the very first matmuls can
    # start as soon as their exact bytes (w cols 0:512 + x rt0 slice) land
    L0 = cuts[0] - Nc
    head_edges = {e for e in (512, L0, L0 + P) if 0 < e < cuts[0]}
    block_edges = sorted(set(block_edges) | head_edges)
    load_plan = []
    prev = 0
    for edge in block_edges[1:]:
        lim = CHUNK1_COLS if edge <= cuts[0] + HEAD_COLS else CHUNK_COLS
        if edge in head_edges or edge == cuts[0] or edge - prev >= lim \
                or edge == in_cols:
            load_plan.append(("in", prev, edge))
            prev = edge
    assert prev == in_cols

    return {
        "N": N, "F": F, "OUT_F": OUT_F, "BS": BS,
        "wslots": dict(wslots),
        "xt_tiles": xt_tiles,
        "in_blocks": in_blocks, "in_cols": in_cols,
        "segments": segments, "out_cols": out_cols,
        "rows_per_core": rows_per_core, "rt_count": rt_count,
        "load_plan": load_plan,
    }


def _build_nc(meta):
    """Raw bacc module: manual semaphores, no TileContext."""
    Nc = meta["rows_per_core"]
    INC = meta["in_cols"]
    OUTC = meta["out_cols"]
    rt_count = meta["rt_count"]
    BS = meta["BS"]
    segs = meta["segments"]
    n_seg = len(segs)
    n_groups = n_seg * rt_count

    n_warm = int(os.environ.get("KWARM", "6"))
    warm_n = int(os.environ.get("KWARMN", "512"))  # cols per warm matmul

    nc = bacc.Bacc("TRN2", target_bir_lowering=False, debug=False)
    in_d = nc.dram_tensor("inp", [P, INC], DT_IN, kind="ExternalInput")
    out_d = nc.dram_tensor("out", [Nc, OUTC], DT_OUT, kind="ExternalOutput")

    inp = nc.alloc_sbuf_tensor("inp_sb", [P, INC], DT_IN)
    outs = [nc.alloc_sbuf_tensor(f"osb{r}", [P, OUTC], DT_OUT)
            for r in range(rt_count)]
    wsb = nc.alloc_sbuf_tensor("wsb", [P, 128 + warm_n], DT_IN)

    ps_cols = max(seg["n_obs"] * BS for seg in segs)
    ps_banks_cols = (ps_cols + 511) // 512 * 512
    n_ps = 8 // (ps_banks_cols // 512)
    psums = [nc.alloc_psum_tensor(f"ps{b}", [P, ps_banks_cols], F32)
             for b in range(n_ps)]

    n_chunks = len(meta["load_plan"])
    # one semaphore per input chunk: a shared counter would be racy across
    # the 16 SDMA queues (an intermediate threshold can be reached by a mix
    # of completions from different chunks)
    s_in = [nc.alloc_semaphore(f"s_in{i}") for i in range(n_chunks)]
    s_ws = nc.alloc_semaphore("s_ws")
    s_mm = nc.alloc_semaphore("s_mm")
    s_evA = nc.alloc_semaphore("s_evA")
    s_evB = nc.alloc_semaphore("s_evB")
    s_st = nc.alloc_semaphore("s_st")
    all_sems = s_in + [s_ws, s_mm, s_evA, s_evB, s_st]
    sem_nums = sorted(s.num for s in all_sems)
    assert sem_nums == list(range(sem_nums[0], sem_nums[0] + len(all_sems)))
    sem_rng = range(sem_nums[0], sem_nums[-1] + 1)

    # optional defensive start-state clear (NRT's own post-execution sweep
    # resets all semaphores, so this is normally redundant)
    if os.environ.get("KSTARTCLR", "0") == "1":
        nc.gpsimd.dma_reset(sem_rng)
        nc.gpsimd.sem_clear(sem_rng)
        nc.all_engine_barrier()

    # ---- eviction plan ----------------------------------------------------
    # group g = si*rt_count + rt.  Groups in the last segment are split
    # half/half across ACT and DVE (short tail); earlier groups alternate.
    # Each eviction item: (g, col_lo, col_hi).  Engine sem counts follow
    # list order.
    split_ev = os.environ.get("KSPLITEV", "0") == "1"
    evA, evB = [], []          # (g, c0, c1)
    for g in range(n_groups):
        si, rt = divmod(g, rt_count)
        L = segs[si]["n_obs"] * BS
        if si == n_seg - 1 and split_ev:
            h = (L // 2 + 1) // 2 * 2
            evA.append((g, 0, h))
            evB.append((g, h, L))
        elif g % 2 == 0:
            evA.append((g, 0, L))
        else:
            evB.append((g, 0, L))
    posA = {g: max(i + 1 for i, (gg, _, _) in enumerate(evA) if gg == g)
            for g in {e[0] for e in evA}}
    posB = {g: max(i + 1 for i, (gg, _, _) in enumerate(evB) if gg == g)
            for g in {e[0] for e in evB}}

    def ev_wait(engine, groups):
        """Wait until the evictions of all `groups` fully finished."""
        if isinstance(groups, int):
            groups = [groups]
        a = max((posA[g] for g in groups if g in posA), default=0)
        b = max((posB[g] for g in groups if g in posB), default=0)
        if a:
            engine.wait_ge(s_evA, a)
        if b:
            engine.wait_ge(s_evB, b)

    # ---- SP: all input loads up front ------------------------------------
    for i, (_, a, b) in enumerate(meta["load_plan"]):
        nc.sync.dma_start(out=inp[:, a:b], in_=in_d[:, a:b]).then_inc(s_in[i], 16)
    chunk_end = [b for (_, a, b) in meta["load_plan"]]

    def chunk_of(col):
        # index of the chunk that contains col-1 (i.e. covers cols < col)
        return bisect_left(chunk_end, col)

    # ---- warmup ----------------------------------------------------------
    nc.gpsimd.memset(wsb[:].bitcast(F32), 0).then_inc(s_ws)
    if n_warm:
        nc.tensor.wait_ge(s_ws, 1)
        for _ in range(n_warm):
            nc.tensor.matmul(psums[-1][:, :warm_n], wsb[:, :P],
                             wsb[:, P:P + warm_n], start=True, stop=True)

    # ---- main pipeline ----------------------------------------------------
    # PE stream (group-major), evictions, stores: emitted in group order so
    # each engine's program order is ascending in its own wait thresholds.
    evA_emit = 0
    evB_emit = 0
    n_stores = 0
    flushed = [0] * rt_count
    waited_chunk = 0
    flush_cols = int(os.environ.get("KFLUSH", "2000"))

    max_g = int(os.environ.get("KMAXG", str(n_groups)))
    for g in range(min(n_groups, max_g)):
        si, rt = divmod(g, rt_count)
        seg = segs[si]
        L = seg["n_obs"] * BS
        ps = psums[g % n_ps]

        # PSUM buffer reuse: wait for eviction of the previous user
        if g >= n_ps:
            ev_wait(nc.tensor, g - n_ps)

        last_mm = len(seg["tasks"]) - 1
        for ti, (c0, c1, lc, rb, kr, wc, start, stop) in enumerate(seg["tasks"]):
            need = max(wc + (c1 - c0), lc + (rt + 1) * P)
            ck = chunk_of(need)
            while waited_chunk <= ck:
                nc.tensor.wait_ge(s_in[waited_chunk], 16)
                waited_chunk += 1
            lhsT = inp[rb:rb + kr, lc + rt * P: lc + (rt + 1) * P]
            mm = nc.tensor.matmul(ps[:, c0:c1], lhsT,
                                  inp[rb:rb + kr, wc:wc + (c1 - c0)],
                                  start=start, stop=stop)
            if ti == last_mm:
                mm.then_inc(s_mm)

        # evictions for this group (ACT and/or DVE)
        dst_base = seg["out_base"]
        while evA_emit < len(evA) and evA[evA_emit][0] == g:
            _, c0, c1 = evA[evA_emit]
            nc.scalar.wait_ge(s_mm, g + 1)
            nc.scalar.copy(outs[rt][:, dst_base + c0:dst_base + c1],
                           ps[:, c0:c1]).then_inc(s_evA)
            evA_emit += 1
        while evB_emit < len(evB) and evB[evB_emit][0] == g:
            _, c0, c1 = evB[evB_emit]
            nc.vector.wait_ge(s_mm, g + 1)
            nc.vector.tensor_copy(out=outs[rt][:, dst_base + c0:dst_base + c1],
                                  in_=ps[:, c0:c1]).then_inc(s_evB)
            evB_emit += 1

        # store when enough columns accumulated for this rt (or near the end)
        done = dst_base + L
        if done - flushed[rt] >= flush_cols or si >= n_seg - 2:
            need = [s2 * rt_count + rt for s2 in range(si + 1)
                    if segs[s2]["out_base"] >= flushed[rt]]
            ev_wait(nc.sync, need)
            nc.sync.dma_start(
                out=out_d[rt * P:(rt + 1) * P, flushed[rt]:done],
                in_=outs[rt][:, flushed[rt]:done]).then_inc(s_st, 16)
            n_stores += 1
            flushed[rt] = done

    # ---- completion -------------------------------------------------------
    nc.sync.wait_ge(s_st, 16 * n_stores)
    nc.all_engine_barrier()
    if os.environ.get("KENDCLR", "0") == "1":
        nc.gpsimd.dma_reset(sem_rng)
        nc.gpsimd.sem_clear(sem_rng)
        nc.all_engine_barrier()

    nc.compile()
    return nc


def _host_tensors(meta, x2, weight):
    """Build per-core combined input arrays (values only)."""
    BS = meta["BS"]
    Nc = meta["rows_per_core"]
    Ntot = Nc * N_CORES

    if x2.shape[0] < Ntot:
        x2 = np.concatenate(
            [x2, np.zeros((Ntot - x2.shape[0], x2.shape[1]), np.float32)], axis=0)

    wsum = {}
    for (ob_ib, ks) in meta["wslots"].items():
        w = weight[ks[0]]
        for k in ks[1:]:
            w = w + weight[k]
        wsum[ob_ib] = np.ascontiguousarray(w, dtype=np.float32)

    base = np.zeros((P, meta["in_cols"]), np.float32)
    for blk in meta["in_blocks"]:
        if blk[1] != "w":
            continue
        col, _, rb, uibs, seg_obs = blk
        for r, ib in enumerate(uibs):
            row0 = rb + r * 64
            for j, ob in enumerate(seg_obs):
                w = wsum.get((ob, ib))
                if w is not None:
                    base[row0:row0 + 64, col + j * BS: col + (j + 1) * BS] = w

    in_all = []
    for c in range(N_CORES):
        xs = x2[c * Nc:(c + 1) * Nc]
        comb = base.copy()
        for blk in meta["in_blocks"]:
            if blk[1] != "x":
                continue
            col, _, t = blk[0], blk[1], blk[2]
            for (rbase, ib) in meta["xt_tiles"][t]:
                comb[rbase:rbase + 64, col:col + Nc] = \
                    xs[:, ib * BS:(ib + 1) * BS].T
        in_all.append(np.ascontiguousarray(comb.astype(NP_IN)))
    return in_all


def kernel(**inputs):
    global LAST_RESULT
    x = np.asarray(inputs["x"], dtype=np.float32)
    weight = np.asarray(inputs["weight"], dtype=np.float32)
    bias = np.asarray(inputs["bias"], dtype=np.float32)
    out_idx = np.asarray(inputs["out_block_idx"]).astype(np.int64)
    in_idx = np.asarray(inputs["in_block_idx"]).astype(np.int64)

    B, S, F = x.shape
    N = B * S
    BS = weight.shape[1]
    OUT_F = bias.shape[0]
    x2 = np.ascontiguousarray(x.reshape(N, F))

    key = (N, F, OUT_F, BS, out_idx.tobytes(), in_idx.tobytes())
    if key not in _CACHE:
        meta = _build_schedule(N, F, OUT_F, BS, out_idx, in_idx)
        nc = _build_nc(meta)
        _CACHE[key] = (nc, meta)
    nc, meta = _CACHE[key]

    in_all = _host_tensors(meta, x2, weight)
    in_maps = [{"inp": in_all[c]} for c in range(N_CORES)]
    try:
        res = bass_utils.run_bass_kernel_spmd(
            nc, in_maps, core_ids=list(range(N_CORES)))
    except Exception:
        res = bass_utils.run_bass_kernel_spmd(
            nc, in_maps, core_ids=list(range(N_CORES)))
    LAST_RESULT = res

    dev = np.concatenate(
        [np.asarray(res.results[c]["out"]).astype(np.float32)
         for c in range(N_CORES)], axis=0)
    dev = dev[:N]

    out = np.zeros((N, OUT_F), np.float32)
    for seg in meta["segments"]:
        b = seg["out_base"]
        for j, ob in enumerate(seg["obs"]):
            out[:, ob * BS:(ob + 1) * BS] = dev[:, b + j * BS: b + (j + 1) * BS]
    if bias.any():
        out += bias
    return out.reshape(B, S, OUT_F)


# revision 23
# speedup vs baseline: 1.0254x; 1.0254x over previous
"""Block-sparse linear kernel for Trainium2 (8 NeuronCores, raw Bass/bacc).

Computes out[n, ob*BS:(ob+1)*BS] += x[n, ib*BS:(ib+1)*BS] @ W[k] for each
nonzero block k with indices (ob, ib), plus bias — data-parallel over the
flattened row dim N across 8 cores (weights replicated, indices baked into
the schedule host-side).

Host-side schedule (same as the Tile baseline):
  - Group input-blocks (ibs) into *families* with identical sets of
    output-blocks (obs); for the canonical every-10th-block pattern the
    families are the 5 residue classes mod 5 (disjoint, no zero fill).
  - Pair ibs within a family: each pair is one K=128 stationary operand
    (two 64-feature x slices, transposed host-side), streaming a
    [128, n_obs*64] stacked-weight moving operand -> full PE utilization.
  - One combined input tensor holds stacked weights and transposed x
    slices in exact consumption order; a single sequential DMA stream
    delivers data just-in-time.

Device module (this version): raw bacc, no TileContext.  The Tile
epilogue (per-semaphore EVSEM chains, ~7.4us with 254 sems) and its
scheduling slack were the dominant overheads in the 44.6us baseline.
Here the whole pipeline is synchronized with 6 manual semaphores:
  s_in   +16 per input DMA chunk (HWDGE completion)
  s_ws   +1 when the warmup operand tile is memset
  s_mm   +1 per PSUM accumulation group finished (PE order)
  s_evA  +1 per ACT eviction        s_evB  +1 per DVE eviction
  s_st   +16 per output store DMA
PE waits on s_in at chunk-boundary tasks and on s_ev* for PSUM buffer
reuse; ACT/DVE evict alternating groups (last segment split half/half
for a short tail); SP issues all input loads up front, then stores as
evictions land.  bf16 in/out as before (rel err ~2.9e-3).
"""

import os
import numpy as np
import ml_dtypes
from bisect import bisect_left
from collections import defaultdict

from concourse import bass_utils, bacc, mybir

N_CORES = 8
P = 128            # partitions / row-tile size
SEG_MAX_OBS = int(os.environ.get("KSEG", "16"))  # blocks per psum segment
F32R = mybir.dt.float32r
F32 = mybir.dt.float32
BF16 = mybir.dt.bfloat16

KDTYPE = os.environ.get("KDTYPE", "bf16")
DT_IN = BF16 if KDTYPE == "bf16" else F32R
NP_IN = ml_dtypes.bfloat16 if KDTYPE == "bf16" else np.float32
KOUT = os.environ.get("KOUT", "bf16")
DT_OUT = BF16 if KOUT == "bf16" else F32
NP_OUT = ml_dtypes.bfloat16 if KOUT == "bf16" else np.float32

_CACHE = {}
LAST_RESULT = None


def _build_schedule(N, F, OUT_F, BS, out_idx, in_idx):
    """Pure-index schedule: families, pairs, segments, layouts."""
    n_ib = F // BS
    n_ob = OUT_F // BS
    assert F % BS == 0 and OUT_F % BS == 0

    wslots = defaultdict(list)
    for k, (ob, ib) in enumerate(zip(out_idx, in_idx)):
        ob, ib = int(ob), int(ib)
        assert 0 <= ob < n_ob and 0 <= ib < n_ib
        wslots[(ob, ib)].append(k)

    obs_by_ib = defaultdict(set)
    for (ob, ib) in wslots:
        obs_by_ib[ib].add(ob)

    fam_map = defaultdict(list)
    for ib in sorted(obs_by_ib):
        fam_map[frozenset(obs_by_ib[ib])].append(ib)
    families = [(sorted(obs), ibs) for obs, ibs in fam_map.items()]

    parent = {}

    def find(a):
        while parent[a] != a:
            parent[a] = parent[parent[a]]
            a = parent[a]
        return a

    for obs, _ in families:
        for ob in obs:
            parent.setdefault(ob, ob)
        r = find(obs[0])
        for ob in obs[1:]:
            parent[find(ob)] = r
    sf_map = defaultdict(lambda: {"obs": set(), "fams": []})
    for obs, ibs in families:
        root = find(obs[0])
        sf_map[root]["obs"].update(obs)
        sf_map[root]["fams"].append((obs, ibs))
    superfams = sorted(sf_map.values(), key=lambda s: min(s["obs"]))

    # order superfamilies: smallest stream first (early PE start), smallest
    # last (short tail); big ones in the middle.
    def sf_cols(sf):
        u = sum((len(ibs) + 1) // 2 for _, ibs in sf["fams"])
        return u * len(sf["obs"]) * BS

    if len(superfams) > 2:
        rest = sorted(superfams, key=sf_cols)
        first, last = rest[1], rest[0]
        mid = rest[2:]
        superfams = [first] + mid + [last]

    xt_tiles = []
    singles = []
    fam_units = defaultdict(list)
    fam_id = 0
    for sf in superfams:
        for obs, ibs in sf["fams"]:
            key = fam_id
            for i in range(0, len(ibs) - 1, 2):
                t = len(xt_tiles)
                xt_tiles.append([(0, ibs[i]), (64, ibs[i + 1])])
                fam_units[key].append((t, 0, 128, (ibs[i], ibs[i + 1])))
            if len(ibs) % 2:
                singles.append((key, ibs[-1]))
            fam_id += 1
    for j in range(0, len(singles), 2):
        t = len(xt_tiles)
        entries = [(0, singles[j][1])]
        fam_units[singles[j][0]].append((t, 0, 64, (singles[j][1],)))
        if j + 1 < len(singles):
            entries.append((64, singles[j + 1][1]))
            fam_units[singles[j + 1][0]].append((t, 64, 64, (singles[j + 1][1],)))
        xt_tiles.append(entries)

    n_pad = (-N) % (N_CORES * P)
    rows_per_core = (N + n_pad) // N_CORES
    rt_count = rows_per_core // P
    Nc = rows_per_core

    # segments + combined-input layout + out layout
    segments = []
    in_blocks = []
    xt_off = {}
    in_cols = 0
    out_cols = 0
    cuts = []
    fid = 0
    for sfi, sf in enumerate(superfams):
        sf_obs = sorted(sf["obs"])
        units = []
        for obs, ibs in sf["fams"]:
            units.append((fid, tuple(obs)))
            fid += 1
        for s0 in range(0, len(sf_obs), SEG_MAX_OBS):
            seg_obs = sf_obs[s0:s0 + SEG_MAX_OBS]
            L = len(seg_obs) * BS
            tasks = []
            all_units = []
            for key, fobs in units:
                for (t, rb, kr, uibs) in fam_units[key]:
                    all_units.append((t, rb, kr, uibs))
            for ui, (t, rb, kr, uibs) in enumerate(all_units):
                wc = in_cols
                in_blocks.append((wc, "w", rb, uibs, seg_obs))
                in_cols += L
                if t not in xt_off:
                    xt_off[t] = in_cols
                    in_blocks.append((in_cols, "x", t, None, None))
                    in_cols += Nc
                for c0 in range(0, L, 512):
                    c1 = min(c0 + 512, L)
                    tasks.append((c0, c1, xt_off[t], rb, kr, wc + c0,
                                  ui == 0, ui == len(all_units) - 1))
                if len(cuts) == 0 and len(segments) == 0 and ui == 0:
                    cuts.append(in_cols)   # first chunk: unit0 (+ its xt)
            segments.append({"out_base": out_cols, "n_obs": len(seg_obs),
                             "obs": seg_obs, "tasks": tasks})
            out_cols += L
    cuts.append(in_cols)

    # chunking: small chunks at the head (the ~1-2us DMA completion receipt
    # latency otherwise stalls the PE while it still tracks the stream),
    # bigger chunks once the PE has fallen behind the load stream.
    CHUNK_COLS = int(os.environ.get("KCHUNK", "3400"))
    CHUNK1_COLS = int(os.environ.get("KCHUNK1", os.environ.get("KCHUNK", "3400")))
    HEAD_COLS = int(os.environ.get("KHEAD", "3000"))
    block_edges = sorted({b[0] for b in in_blocks} | {in_cols})
    load_plan = []
    prev = 0
    for edge in block_edges[1:]:
        lim = CHUNK1_COLS if edge <= cuts[0] + HEAD_COLS else CHUNK_COLS
        if edge == cuts[0] or edge - prev >= lim or edge == in_cols:
            load_plan.append(("in", prev, edge))
            prev = edge
    assert prev == in_cols

    return {
        "N": N, "F": F, "OUT_F": OUT_F, "BS": BS,
        "wslots": dict(wslots),
        "xt_tiles": xt_tiles,
        "in_blocks": in_blocks, "in_cols": in_cols,
        "segments": segments, "out_cols": out_cols,
        "rows_per_core": rows_per_core, "rt_count": rt_count,
        "load_plan": load_plan,
    }


def _build_nc(meta):
    """Raw bacc module: manual semaphores, no TileContext."""
    Nc = meta["rows_per_core"]
    INC = meta["in_cols"]
    OUTC = meta["out_cols"]
    rt_count = meta["rt_count"]
    BS = meta["BS"]
    segs = meta["segments"]
    n_seg = len(segs)
    n_groups = n_seg * rt_count

    n_warm = int(os.environ.get("KWARM", "6"))
    warm_n = int(os.environ.get("KWARMN", "512"))  # cols per warm matmul

    nc = bacc.Bacc("TRN2", target_bir_lowering=False, debug=False)
    in_d = nc.dram_tensor("inp", [P, INC], DT_IN, kind="ExternalInput")
    out_d = nc.dram_tensor("out", [Nc, OUTC], DT_OUT, kind="ExternalOutput")

    inp = nc.alloc_sbuf_tensor("inp_sb", [P, INC], DT_IN)
    outs = [nc.alloc_sbuf_tensor(f"osb{r}", [P, OUTC], DT_OUT)
            for r in range(rt_count)]
    wsb = nc.alloc_sbuf_tensor("wsb", [P, 128 + warm_n], DT_IN)

    ps_cols = max(seg["n_obs"] * BS for seg in segs)
    ps_banks_cols = (ps_cols + 511) // 512 * 512
    n_ps = 8 // (ps_banks_cols // 512)
    psums = [nc.alloc_psum_tensor(f"ps{b}", [P, ps_banks_cols], F32)
             for b in range(n_ps)]

    n_chunks = len(meta["load_plan"])
    # one semaphore per input chunk: a shared counter would be racy across
    # the 16 SDMA queues (an intermediate threshold can be reached by a mix
    # of completions from different chunks)
    s_in = [nc.alloc_semaphore(f"s_in{i}") for i in range(n_chunks)]
    s_ws = nc.alloc_semaphore("s_ws")
    s_mm = nc.alloc_semaphore("s_mm")
    s_evA = nc.alloc_semaphore("s_evA")
    s_evB = nc.alloc_semaphore("s_evB")
    s_st = nc.alloc_semaphore("s_st")
    all_sems = s_in + [s_ws, s_mm, s_evA, s_evB, s_st]
    sem_nums = sorted(s.num for s in all_sems)
    assert sem_nums == list(range(sem_nums[0], sem_nums[0] + len(all_sems)))
    sem_rng = range(sem_nums[0], sem_nums[-1] + 1)

    # optional defensive start-state clear (NRT's own post-execution sweep
    # resets all semaphores, so this is normally redundant)
    if os.environ.get("KSTARTCLR", "0") == "1":
        nc.gpsimd.dma_reset(sem_rng)
        nc.gpsimd.sem_clear(sem_rng)
        nc.all_engine_barrier()

    # ---- eviction plan ----------------------------------------------------
    # group g = si*rt_count + rt.  Groups in the last segment are split
    # half/half across ACT and DVE (short tail); earlier groups alternate.
    # Each eviction item: (g, col_lo, col_hi).  Engine sem counts follow
    # list order.
    split_ev = os.environ.get("KSPLITEV", "0") == "1"
    evA, evB = [], []          # (g, c0, c1)
    for g in range(n_groups):
        si, rt = divmod(g, rt_count)
        L = segs[si]["n_obs"] * BS
        if si == n_seg - 1 and split_ev:
            h = (L // 2 + 1) // 2 * 2
            evA.append((g, 0, h))
            evB.append((g, h, L))
        elif g % 2 == 0:
            evA.append((g, 0, L))
        else:
            evB.append((g, 0, L))
    posA = {g: max(i + 1 for i, (gg, _, _) in enumerate(evA) if gg == g)
            for g in {e[0] for e in evA}}
    posB = {g: max(i + 1 for i, (gg, _, _) in enumerate(evB) if gg == g)
            for g in {e[0] for e in evB}}

    def ev_wait(engine, groups):
        """Wait until the evictions of all `groups` fully finished."""
        if isinstance(groups, int):
            groups = [groups]
        a = max((posA[g] for g in groups if g in posA), default=0)
        b = max((posB[g] for g in groups if g in posB), default=0)
        if a:
            engine.wait_ge(s_evA, a)
        if b:
            engine.wait_ge(s_evB, b)

    # ---- input loads up front, alternating both HWDGE rings (SP + ACT) ---
    # trigger issue costs ~0.65us of sequencer time per chunk; two rings
    # halve the serialization so the bulk stream starts sooner
    dual = os.environ.get("KDUAL", "1") == "1"
    for i, (_, a, b) in enumerate(meta["load_plan"]):
        eng = nc.scalar if (dual and i % 2 == 1) else nc.sync
        eng.dma_start(out=inp[:, a:b], in_=in_d[:, a:b]).then_inc(s_in[i], 16)
    chunk_end = [b for (_, a, b) in meta["load_plan"]]

    def chunk_of(col):
        # index of the chunk that contains col-1 (i.e. covers cols < col)
        return bisect_left(chunk_end, col)

    # ---- warmup ----------------------------------------------------------
    nc.vector.memset(wsb[:].bitcast(F32), 0).then_inc(s_ws)
    if n_warm:
        nc.tensor.wait_ge(s_ws, 1)
        for _ in range(n_warm):
            nc.tensor.matmul(psums[-1][:, :warm_n], wsb[:, :P],
                             wsb[:, P:P + warm_n], start=True, stop=True)

    # ---- main pipeline ----------------------------------------------------
    # PE stream (group-major), evictions, stores: emitted in group order so
    # each engine's program order is ascending in its own wait thresholds.
    evA_emit = 0
    evB_emit = 0
    n_stores = 0
    flushed = [0] * rt_count
    waited_chunk = 0
    flush_cols = int(os.environ.get("KFLUSH", "2000"))

    max_g = int(os.environ.get("KMAXG", str(n_groups)))
    for g in range(min(n_groups, max_g)):
        si, rt = divmod(g, rt_count)
        seg = segs[si]
        L = seg["n_obs"] * BS
        ps = psums[g % n_ps]

        # PSUM buffer reuse: wait for eviction of the previous user
        if g >= n_ps:
            ev_wait(nc.tensor, g - n_ps)

        last_mm = len(seg["tasks"]) - 1
        for ti, (c0, c1, lc, rb, kr, wc, start, stop) in enumerate(seg["tasks"]):
            need = max(wc + (c1 - c0), lc + (rt + 1) * P)
            ck = chunk_of(need)
            while waited_chunk <= ck:
                nc.tensor.wait_ge(s_in[waited_chunk], 16)
                waited_chunk += 1
            lhsT = inp[rb:rb + kr, lc + rt * P: lc + (rt + 1) * P]
            mm = nc.tensor.matmul(ps[:, c0:c1], lhsT,
                                  inp[rb:rb + kr, wc:wc + (c1 - c0)],
                                  start=start, stop=stop)
            if ti == last_mm:
                mm.then_inc(s_mm)

        # evictions for this group (ACT and/or DVE)
        dst_base = seg["out_base"]
        while evA_emit < len(evA) and evA[evA_emit][0] == g:
            _, c0, c1 = evA[evA_emit]
            nc.scalar.wait_ge(s_mm, g + 1)
            nc.scalar.copy(outs[rt][:, dst_base + c0:dst_base + c1],
                           ps[:, c0:c1]).then_inc(s_evA)
            evA_emit += 1
        while evB_emit < len(evB) and evB[evB_emit][0] == g:
            _, c0, c1 = evB[evB_emit]
            nc.vector.wait_ge(s_mm, g + 1)
            nc.vector.tensor_copy(out=outs[rt][:, dst_base + c0:dst_base + c1],
                                  in_=ps[:, c0:c1]).then_inc(s_evB)
            evB_emit += 1

        # store when enough columns accumulated for this rt (or near the end)
        done = dst_base + L
        if done - flushed[rt] >= flush_cols or si >= n_seg - 2:
            need = [s2 * rt_count + rt for s2 in range(si + 1)
                    if segs[s2]["out_base"] >= flushed[rt]]
            ev_wait(nc.sync, need)
            nc.sync.dma_start(
                out=out_d[rt * P:(rt + 1) * P, flushed[rt]:done],
                in_=outs[rt][:, flushed[rt]:done]).then_inc(s_st, 16)
            n_stores += 1
            flushed[rt] = done

    # ---- completion -------------------------------------------------------
    nc.sync.wait_ge(s_st, 16 * n_stores)
    nc.all_engine_barrier()
    if os.environ.get("KENDCLR", "0") == "1":
        nc.gpsimd.dma_reset(sem_rng)
        nc.gpsimd.sem_clear(sem_rng)
        nc.all_engine_barrier()

    nc.compile()
    return nc


def _host_tensors(meta, x2, weight):
    """Build per-core combined input arrays (values only)."""
    BS = meta["BS"]
    Nc = meta["rows_per_core"]
    Ntot = Nc * N_CORES

    if x2.shape[0] < Ntot:
        x2 = np.concatenate(
            [x2, np.zeros((Ntot - x2.shape[0], x2.shape[1]), np.float32)], axis=0)

    wsum = {}
    for (ob_ib, ks) in meta["wslots"].items():
        w = weight[ks[0]]
        for k in ks[1:]:
            w = w + weight[k]
        wsum[ob_ib] = np.ascontiguousarray(w, dtype=np.float32)

    base = np.zeros((P, meta["in_cols"]), np.float32)
    for blk in meta["in_blocks"]:
        if blk[1] != "w":
            continue
        col, _, rb, uibs, seg_obs = blk
        for r, ib in enumerate(uibs):
            row0 = rb + r * 64
            for j, ob in enumerate(seg_obs):
                w = wsum.get((ob, ib))
                if w is not None:
                    base[row0:row0 + 64, col + j * BS: col + (j + 1) * BS] = w

    in_all = []
    for c in range(N_CORES):
        xs = x2[c * Nc:(c + 1) * Nc]
        comb = base.copy()
        for blk in meta["in_blocks"]:
            if blk[1] != "x":
                continue
            col, _, t = blk[0], blk[1], blk[2]
            for (rbase, ib) in meta["xt_tiles"][t]:
                comb[rbase:rbase + 64, col:col + Nc] = \
                    xs[:, ib * BS:(ib + 1) * BS].T
        in_all.append(np.ascontiguousarray(comb.astype(NP_IN)))
    return in_all


def kernel(**inputs):
    global LAST_RESULT
    x = np.asarray(inputs["x"], dtype=np.float32)
    weight = np.asarray(inputs["weight"], dtype=np.float32)
    bias = np.asarray(inputs["bias"], dtype=np.float32)
    out_idx = np.asarray(inputs["out_block_idx"]).astype(np.int64)
    in_idx = np.asarray(inputs["in_block_idx"]).astype(np.int64)

    B, S, F = x.shape
    N = B * S
    BS = weight.shape[1]
    OUT_F = bias.shape[0]
    x2 = np.ascontiguousarray(x.reshape(N, F))

    key = (N, F, OUT_F, BS, out_idx.tobytes(), in_idx.tobytes())
    if key not in _CACHE:
        meta = _build_schedule(N, F, OUT_F, BS, out_idx, in_idx)
        nc = _build_nc(meta)
        _CACHE[key] = (nc, meta)
    nc, meta = _CACHE[key]

    in_all = _host_tensors(meta, x2, weight)
    in_maps = [{"inp": in_all[c]} for c in range(N_CORES)]
    try:
        res = bass_utils.run_bass_kernel_spmd(
            nc, in_maps, core_ids=list(range(N_CORES)))
    except Exception:
        res = bass_utils.run_bass_kernel_spmd(
            nc, in_maps, core_ids=list(range(N_CORES)))
    LAST_RESULT = res

    dev = np.concatenate(
        [np.asarray(res.results[c]["out"]).astype(np.float32)
         for c in range(N_CORES)], axis=0)
    dev = dev[:N]

    out = np.zeros((N, OUT_F), np.float32)
    for seg in meta["segments"]:
        b = seg["out_base"]
        for j, ob in enumerate(seg["obs"]):
            out[:, ob * BS:(ob + 1) * BS] = dev[:, b + j * BS: b + (j + 1) * BS]
    if bias.any():
        out += bias
    return out.reshape(B, S, OUT_F)
